# revision 3
# baseline (speedup 1.0000x reference)
"""NetVLAD Trainium2 kernel v4 — fp16 datapath, per-image XBAR transposes,
1024-px chunks, batched softmax pipeline, single activation table.

Host ships x as fp16 [NIMG, C, P] (plus img0 quarters for fast start).
Per image ONE DMA transpose: xts[p, t, c] = x[c, t*128+p] (aligned 256
stride, col 128 later holds n for the vlad s-column).
b folds into eb' = exp(b - bmid - theta) (fp32, tiled x8); theta rides the
batched exp as a float bias; all per-pixel scalars apply as broadcast
tensor_tensor ops over whole 1024-px chunks.

Per 1024-px chunk (8 px-tiles):
  PE    8 u-matmuls: psU[:, j, :] = xb_j.T @ wT   ([128,8,64] fp32, 1 bank)
  ACT   sq = xts^2 (f16) ; DVE: ssq = reduce(sq), negMu = -max_k(u)
  ACT   ln, invc = exp(-.5 ln ssq) ; Pool: t5 = negMu*invc
  DVE   ls = u * invcB ; ls2 = ls + t5B     (fp16)
  ACT   E = exp(ls2 + theta) -> bf16        (one op per chunk)
  Pool  EB = E * eb8 ; DVE: scol = reduce_k EB, gcol = 1/scol
  Pool  rcol = invc*gcol ; Ep = EB * rcolB ; ncol = ssq*invc
  DVE   xts[:, t, 128] = ncol
  PE    psV[56, 0:129] += Ep[:, j, 0:56].T @ xts[:, j, 0:129]  (lag 3)
"""

import sys

for _p in ("/opt/trn_rl_repo",):
    if _p not in sys.path:
        sys.path.insert(0, _p)

import numpy as np

NIMG = 4
C = 128
K = 64
KE = 56
P = 4096
NCH = 4        # 1024-px chunks per image
TPC = 8        # 128-px tiles per chunk
NSLOT = NIMG * NCH
LAG = 3
THETA = 35.0

DBG = False

_cache = {}


def _build():
    import concourse.mybir as mybir
    from concourse import bacc, tile
    from concourse.hw_specs import get_activation_tables

    f32 = mybir.dt.float32
    f16 = mybir.dt.float16
    bf16 = mybir.dt.bfloat16
    Alu = mybir.AluOpType
    Act = mybir.ActivationFunctionType

    nc = bacc.Bacc()
    xcp_in = nc.declare_dram_parameter("xcp", [NIMG, C, P], f16, isOutput=False)
    xq_in = nc.declare_dram_parameter("xq0", [4, C, 1024], f16, isOutput=False)
    cw_in = nc.declare_dram_parameter("cw", [C, K], f16, isOutput=False)
    cb_in = nc.declare_dram_parameter("cb", [C, C], f32, isOutput=False)
    # f32: [0:1 ones | 1:513 eb8 | 513:641 cen r0:56 | p0 650:778 ones-row]
    cf_in = nc.declare_dram_parameter("cf", [C, 778], f32, isOutput=False)
    out_ext = nc.declare_dram_parameter("out", [NIMG, KE, C], f32,
                                        isOutput=True)
    dbg_ext = nc.declare_dram_parameter("dbg", [C, 1024], f32, isOutput=True)

    tabs = list(get_activation_tables(nc.m.arch).keys())
    tab_id = tabs.index("natural_log_exp_and_others")

    with tile.TileContext(nc) as tc:
        with (
            tc.tile_pool(name="const", bufs=1) as cpool,
            tc.tile_pool(name="xq", bufs=4) as xqpool,
            tc.tile_pool(name="xb", bufs=3) as xbpool,
            tc.tile_pool(name="xts", bufs=3) as tpool,
            tc.tile_pool(name="ls", bufs=2) as lpool,
            tc.tile_pool(name="ew", bufs=3) as epool,
            tc.tile_pool(name="sq", bufs=2) as qpool,
            tc.tile_pool(name="stats", bufs=4) as spool,
            tc.tile_pool(name="fin", bufs=2) as fpool,
            tc.tile_pool(name="psU", bufs=3, space="PSUM") as pU,
            tc.tile_pool(name="psV", bufs=2, space="PSUM") as pV,
            tc.tile_pool(name="psT", bufs=2, space="PSUM") as pT,
        ):
            nc.scalar.add_instruction(mybir.InstLoadActFuncSet(
                name=nc.get_next_instruction_name(), ins=[], outs=[],
                act_func_set_id=tab_id))

            cw16 = cpool.tile([C, K], f16, tag="cw16")
            nc.sync.dma_start(cw16[:], cw_in[:])
            xts = {}
            xts[0] = tpool.tile([C, 32, 256], f16, tag="xts", name="xts0")
            for q in range(4):
                nc.sync.dma_start_transpose(
                    xts[0][:, q * 8:(q + 1) * 8, 0:C], xq_in[q])
            cbi = cpool.tile([C, C], bf16, tag="cbi")
            nc.gpsimd.dma_start(cbi[:], cb_in[:])
            cf32 = cpool.tile([C, 778], f32, tag="cf32")
            nc.sync.dma_start(cf32[:], cf_in[:])
            onescol = cf32[:, 0:1]
            eb8 = cf32[:, 1:513]
            cen = cf32[0:KE, 513:641]
            onesrow = cf32[0:1, 650:778]
            thetacol = cf32[:, 641:642]
            ident_b = cbi[:, 0:C]
            ident56_b = cbi[0:KE, 0:KE]

            # x [c,p] SBUF for u-matmul stationary
            xq = []
            for q in range(4):
                t = xqpool.tile([C, P // 4], f16, tag="xq", name="xqt")
                nc.sync.dma_start(t[:], xcp_in[0, :, q * 1024:(q + 1) * 1024])
                xq.append(t)
            xb = {}
            xb[1] = xbpool.tile([C, P], f16, tag="xb", name="xbt")
            nc.sync.dma_start(xb[1][:], xcp_in[1])

            xts[1] = tpool.tile([C, 32, 256], f16, tag="xts", name="xts1")
            nc.sync.dma_start_transpose(xts[1][:, :, 0:C], xcp_in[1])

            def xsrc(c, j):
                img, ch = divmod(c, NCH)
                base = ch * 1024 + j * 128
                if img == 0:
                    return xq[base // 1024][:, base % 1024:base % 1024 + 128]
                return xb[img][:, base:base + 128]

            psU = {}
            lst = {}
            et = {}
            ebt = {}
            ept = {}
            st = {}
            psV = {}
            vk = {}
            tailseq = []

            def mms(c):
                img = c // NCH
                if c % NCH == 0 and img + 1 < NIMG and img + 1 not in xb:
                    xb[img + 1] = xbpool.tile([C, P], f16, tag="xb",
                                              name="xbt")
                    nc.sync.dma_start(xb[img + 1][:], xcp_in[img + 1])
                    xts[img + 1] = tpool.tile([C, 32, 256], f16, tag="xts",
                                              name="xtst")
                    nc.sync.dma_start_transpose(xts[img + 1][:, :, 0:C],
                                                xcp_in[img + 1])
                psU[c] = pU.tile([C, TPC, K], f32, tag="psU", name="psUt")
                for j in range(TPC):
                    nc.tensor.matmul(psU[c][:, j:j + 1, :], xsrc(c, j),
                                     cw16[:], start=True, stop=True)

            def xv(c):
                img, ch = divmod(c, NCH)
                return xts[img][:, ch * TPC:(ch + 1) * TPC, :]

            def stats_a(c):
                s = {}
                sq = qpool.tile([C, TPC * C], f16, tag="sqt")
                nc.scalar.activation(
                    sq[:].rearrange("p (t q) -> p t q", q=C),
                    xv(c)[:, :, 0:C], Act.Square)
                s["ssq"] = spool.tile([C, TPC], f32, tag="ssq", name="ssq")
                nc.vector.tensor_reduce(
                    s["ssq"][:], sq[:].rearrange("p (t q) -> p t q", q=C),
                    axis=mybir.AxisListType.X, op=Alu.add)
                st[c] = s

            def stats_b(c):
                s = st[c]
                s["lssq"] = spool.tile([C, TPC], f32, tag="lssq", name="lssq")
                nc.scalar.activation(s["lssq"][:], s["ssq"][:], Act.Ln)
                s["invc"] = spool.tile([C, TPC], f32, tag="invc", name="invc")
                nc.scalar.activation(s["invc"][:], s["lssq"][:], Act.Exp,
                                     scale=-0.5)
                ls = lpool.tile([C, TPC * K], f16, tag="ls", name="ls")
                nc.vector.tensor_tensor(
                    ls[:].rearrange("p (t k) -> p t k", k=K), psU[c][:, :, :],
                    s["invc"][:].broadcast_to([C, TPC, K]), Alu.mult)
                s["negm"] = spool.tile([C, TPC], f32, tag="negm", name="negm")
                nc.vector.tensor_reduce(
                    s["negm"][:], ls[:].rearrange("p (t k) -> p t k", k=K),
                    axis=mybir.AxisListType.X, op=Alu.max, negate=True)
                ls2 = lpool.tile([C, TPC * K], f16, tag="ls2", name="ls2")
                nc.vector.tensor_tensor(
                    ls2[:].rearrange("p (t k) -> p t k", k=K),
                    ls[:].rearrange("p (t k) -> p t k", k=K),
                    s["negm"][:].broadcast_to([C, TPC, K]), Alu.add)
                lst[c] = ls2
                et[c] = epool.tile([C, TPC * K], bf16, tag="E", name="et")
                nc.scalar.activation(et[c][:], ls2[:], Act.Exp,
                                     bias=thetacol)

            def stats_c(c):
                s = st[c]
                ebt[c] = epool.tile([C, TPC * K], bf16, tag="EB", name="ebt")
                nc.gpsimd.tensor_tensor(ebt[c][:], et[c][:], eb8, Alu.mult)
                s["scol"] = spool.tile([C, TPC], f32, tag="scol", name="scol")
                nc.vector.tensor_reduce(
                    s["scol"][:], ebt[c][:].rearrange("p (t k) -> p t k", k=K),
                    axis=mybir.AxisListType.X, op=Alu.add)
                s["gcol"] = spool.tile([C, TPC], f32, tag="gcol", name="gcol")
                nc.vector.reciprocal(s["gcol"][:], s["scol"][:])
                s["ncol"] = spool.tile([C, TPC], f32, tag="ncol", name="ncol")
                nc.gpsimd.tensor_tensor(s["ncol"][:], s["ssq"][:],
                                        s["invc"][:], Alu.mult)
                s["rcol"] = spool.tile([C, TPC], f32, tag="rcol", name="rcol")
                nc.gpsimd.tensor_tensor(s["rcol"][:], s["invc"][:],
                                        s["gcol"][:], Alu.mult)
                ept[c] = epool.tile([C, TPC * K], bf16, tag="Ep", name="ept")
                nc.gpsimd.tensor_tensor(
                    ept[c][:].rearrange("p (t k) -> p t k", k=K),
                    ebt[c][:].rearrange("p (t k) -> p t k", k=K),
                    s["rcol"][:].broadcast_to([C, TPC, K]), Alu.mult)
                nc.gpsimd.tensor_tensor(
                    xv(c)[:, :, 128:129],
                    s["ncol"][:].broadcast_to([C, TPC, 1]),
                    onescol.broadcast_to([C, TPC, 1]), Alu.mult)

            def vlads(c):
                img, ch = divmod(c, NCH)
                if ch == 0:
                    psV[img] = pV.tile([KE, 132], f32, tag="psV", name="psVt")
                ev = ept[c][:].rearrange("p (t k) -> p t k", k=K)
                for j in range(TPC):
                    nc.tensor.matmul(psV[img][0:KE, 0:129],
                                     ev[:, j:j + 1, 0:KE],
                                     xv(c)[:, j, 0:129],
                                     start=(ch == 0 and j == 0),
                                     stop=(ch == NCH - 1 and j == TPC - 1))

            def tail_a(img):
                pv = psV[img]
                negs = spool.tile([KE, 1], f32, tag="negs")
                nc.vector.tensor_scalar_mul(negs[:], pv[0:KE, 128:129], -1.0)
                vk[img] = fpool.tile([KE, C], bf16, tag="vk", name="vkt")
                nc.vector.scalar_tensor_tensor(vk[img][:], cen, negs[:],
                                               pv[0:KE, 0:C],
                                               Alu.mult, Alu.add)

            def tail_b(img):
                tt = pT.tile([C, 2, 192], f32, tag="pst", name="pst")
                t1 = tt[:, 0:1, 0:KE]
                nc.tensor.matmul(t1, vk[img][:], ident56_b,
                                 start=True, stop=True)
                tr56 = spool.tile([C, KE], bf16, tag="tr56")
                ssqk = spool.tile([C, 1], f32, tag="ssqk")
                nc.scalar.activation(tr56[:], t1, Act.Square,
                                     accum_out=ssqk[:])
                ssqc = spool.tile([C, 1], f32, tag="ssqc")
                nc.vector.tensor_scalar_max(ssqc[:], ssqk[:], 1e-24)
                lk = spool.tile([C, 1], f32, tag="lk")
                nc.scalar.activation(lk[:], ssqc[:], Act.Ln)
                invk = spool.tile([C, 1], f32, tag="invk")
                nc.scalar.activation(invk[:], lk[:], Act.Exp, scale=-0.5)
                t2 = spool.tile([C, 1], f32, tag="t2")
                nc.vector.scalar_tensor_tensor(t2[:], ssqc[:], invk[:],
                                               invk[:], Alu.mult, Alu.mult)
                tot = spool.tile([1, 1], f32, tag="tot")
                nc.gpsimd.tensor_reduce(tot[:], t2[:],
                                        axis=mybir.AxisListType.C, op=Alu.add)
                totc = spool.tile([1, 1], f32, tag="totc")
                nc.vector.tensor_scalar_max(totc[:], tot[:], 1e-24)
                ltot = spool.tile([1, 1], f32, tag="ltot")
                nc.scalar.activation(ltot[:], totc[:], Act.Ln)
                fv = spool.tile([1, 1], f32, tag="fv")
                nc.scalar.activation(fv[:], ltot[:], Act.Exp, scale=-0.5)
                nc.tensor.matmul(tt[:, 1:2, 188:189], onesrow, fv[:],
                                 start=True, stop=True)
                comb = spool.tile([C, 1], f32, tag="comb")
                nc.vector.tensor_tensor(comb[:], invk[:],
                                        tt[:, 1:2, 188:189], Alu.mult)
                vnT = fpool.tile([C, KE], bf16, tag="vnT", name="vnT")
                nc.vector.tensor_scalar(vnT[:], t1, comb[:], None, Alu.mult)
                return tt, vnT

            def tail_c(img, tt, vnT):
                nc.tensor.matmul(tt[0:KE, 1:2, 0:C], vnT[:], ident_b,
                                 start=True, stop=True)
                ob = fpool.tile([KE, C], f32, tag="ob", name="ob")
                nc.scalar.activation(ob[:], tt[0:KE, 1:2, 0:C], Act.Copy)
                nc.sync.dma_start(out_ext[img], ob[:])

            def dump(c):
                s = st[c]
                nc.gpsimd.dma_start(dbg_ext[:, 0:512], et[c][:])
                nc.sync.dma_start(dbg_ext[:, 512:520], s["ssq"][:])
                nc.sync.dma_start(dbg_ext[:, 520:528], s["invc"][:])
                nc.sync.dma_start(dbg_ext[:, 528:536], s["negm"][:])
                nc.sync.dma_start(dbg_ext[:, 536:544], s["scol"][:])
                nc.sync.dma_start(dbg_ext[:, 544:552], s["rcol"][:])
                nc.sync.dma_start(dbg_ext[:, 552:560], s["ncol"][:])
                nc.gpsimd.dma_start(dbg_ext[:, 560:816], xts[0][:, 0, 0:256])

            for sl in range(NSLOT + LAG + 2):
                while tailseq and tailseq[0][0] <= sl:
                    tailseq.pop(0)[1]()
                if sl < NSLOT:
                    mms(sl)
                v = sl - LAG
                if 0 <= v < NSLOT:
                    vlads(v)
                    img, ch = divmod(v, NCH)
                    if ch == NCH - 1:
                        tail_a(img)

                        def _mk(i):
                            def _b():
                                tt, vnT = tail_b(i)
                                tailseq.append(
                                    (sl + 3, lambda: tail_c(i, tt, vnT)))
                            return _b
                        tailseq.append((sl + 1, _mk(img)))
                if sl < NSLOT:
                    stats_a(sl)
                if 0 <= sl - 1 < NSLOT:
                    stats_b(sl - 1)
                if 0 <= sl - 2 < NSLOT:
                    stats_c(sl - 2)
                    if DBG and sl - 2 == 0:
                        dump(0)
            while tailseq:
                tailseq.pop(0)[1]()

    nc.compile()
    return nc


def _get_nc():
    if "nc" not in _cache:
        _cache["nc"] = _build()
    return _cache["nc"]


def _make_in_maps(x, conv_w, conv_b, centroids):
    x = np.asarray(x, dtype=np.float32)
    conv_w = np.asarray(conv_w, dtype=np.float32)
    conv_b = np.asarray(conv_b, dtype=np.float32)
    centroids = np.asarray(centroids, dtype=np.float32)

    N = x.shape[0]
    n_cores = 8
    per = N // n_cores
    assert per == NIMG

    xr = x.reshape(N, C, P).astype(np.float16)
    bmid = (conv_b.max() + conv_b.min()) / 2.0
    eb = np.exp(conv_b - bmid - THETA).astype(np.float32)

    cf = np.zeros((C, 778), dtype=np.float32)
    cf[:, 0] = 1.0
    cf[:, 1:513] = np.tile(eb, TPC)[None, :]
    cf[0:KE, 513:641] = centroids[:KE]
    cf[:, 641] = THETA
    cf[0, 650:778] = 1.0
    cb = np.eye(C, dtype=np.float32)
    cw = conv_w.T.astype(np.float16)

    in_maps = []
    for i in range(n_cores):
        xc = np.ascontiguousarray(xr[i * per:(i + 1) * per])
        xq0 = np.ascontiguousarray(
            xc[0].reshape(C, 4, 1024).transpose(1, 0, 2))
        in_maps.append({
            "xcp": xc,
            "xq0": xq0,
            "cw": cw,
            "cb": cb,
            "cf": cf,
        })
    return in_maps


def kernel(x, conv_w, conv_b, centroids):
    from concourse.bass_utils import run_bass_kernel_spmd

    in_maps = _make_in_maps(x, conv_w, conv_b, centroids)
    nc = _get_nc()
    res = run_bass_kernel_spmd(nc, in_maps, list(range(8)))
    outs = [np.asarray(r["out"]).reshape(NIMG, KE * C) for r in res.results]
    return np.concatenate(outs, axis=0)


if __name__ == "__main__":
    rng = np.random.default_rng(0)
    x = rng.standard_normal((32, C, 64, 64), dtype=np.float32)
    w = rng.standard_normal((K, C), dtype=np.float32)
    b = rng.standard_normal((K,), dtype=np.float32)
    c = rng.random((K, C), dtype=np.float32)
    out = kernel(x=x, conv_w=w, conv_b=b, centroids=c)
    print(out.shape, out.dtype)
